# revision 1
# baseline (speedup 1.0000x reference)
"""PatchSelfAttentionBlock kernel for Trainium2 (8 NeuronCores, data-parallel over batch).

Per core (one batch element): x[512,1024] +2D-sinusoidal PE, QKV projections,
8-head softmax attention over 1024 tokens, output projection + bias.

Device strategy (bf16 matmuls, fp32 PSUM accumulation):
  - Weights are host-transposed/cast so no on-device transposes are needed.
  - Scores are computed transposed (S^T = K^T Q, tokens-on-partitions), so the
    exp output feeds the PV matmul directly as the moving operand.
  - Every stationary operand serves two back-to-back 512-wide matmuls (query
    chunks), halving weight loads and letting the pair pipeline on the array.
  - S^T tiles span two PSUM banks so each exp is one wide [128,1024] ACT op.
  - V^T carries an appended ones column per head, so the softmax denominator
    falls out of the PV matmul as PSUM row 64.
  - Normalization: per-pair denominators, fast reciprocal, partition-broadcast
    by bouncing through DRAM (step-0 source APs are only legal from DRAM).
"""

import math
import sys

sys.path.insert(0, "/opt/trn_rl_repo")

import numpy as np
import ml_dtypes

import concourse.bass as bass
import concourse.mybir as mybir
import concourse.tile as tile
from concourse import bacc
from concourse.bass_utils import run_bass_kernel_spmd

B, C, H, W = 8, 512, 32, 32
N = H * W          # 1024 tokens
NH = 8             # heads
D = 64             # head dim
CT = C // 128      # 4 channel tiles
MT = N // 128      # 8 token tiles (keys)
NC_ = 512          # query chunk size (one PSUM bank of fp32)
SCALE = 1.0 / math.sqrt(D)

F32 = mybir.dt.float32
BF16 = mybir.dt.bfloat16


def _pe_table():
    half = C // 2
    div = np.exp(np.arange(0, half, 2, dtype=np.float32) * (-math.log(10000.0) / half))
    pos_h = np.arange(H, dtype=np.float32)[:, None] * div[None, :]
    pos_w = np.arange(W, dtype=np.float32)[:, None] * div[None, :]
    emb_h = np.zeros((half, H), dtype=np.float32)
    emb_h[0::2] = np.sin(pos_h).T
    emb_h[1::2] = np.cos(pos_h).T
    emb_w = np.zeros((half, W), dtype=np.float32)
    emb_w[0::2] = np.sin(pos_w).T
    emb_w[1::2] = np.cos(pos_w).T
    pe = np.concatenate(
        [
            np.broadcast_to(emb_h[:, :, None], (half, H, W)),
            np.broadcast_to(emb_w[:, None, :], (half, H, W)),
        ],
        axis=0,
    )
    return np.ascontiguousarray(pe.reshape(C, N))


def _build_program():
    nc = bacc.Bacc("TRN2", target_bir_lowering=False, debug=False, num_devices=B)

    x_ext = nc.dram_tensor("x", [C, N], BF16, kind="ExternalInput").ap()
    pe_ext = nc.dram_tensor("pe", [C, N], BF16, kind="ExternalInput").ap()
    em_ext = nc.dram_tensor("ematrix", [2, 128], F32, kind="ExternalInput").ap()
    wqT_ext = nc.dram_tensor("wqT", [C, C], BF16, kind="ExternalInput").ap()
    wkT_ext = nc.dram_tensor("wkT", [C, C], BF16, kind="ExternalInput").ap()
    wvT_ext = nc.dram_tensor("wvT", [C, C], BF16, kind="ExternalInput").ap()
    woT_ext = nc.dram_tensor("woT", [C, C], BF16, kind="ExternalInput").ap()
    bo_ext = nc.dram_tensor("bo", [C, 1], F32, kind="ExternalInput").ap()
    y_ext = nc.dram_tensor("y", [C, N], F32, kind="ExternalOutput").ap()

    with tile.TileContext(nc) as tc:
        with (
            tc.tile_pool(name="consts", bufs=1) as consts,
            tc.tile_pool(name="xin", bufs=1) as xin_pool,
            tc.tile_pool(name="xpe", bufs=1) as xpe_pool,
            tc.tile_pool(name="qk", bufs=1) as qk_pool,
            tc.tile_pool(name="vt", bufs=1) as vt_pool,
            tc.tile_pool(name="e", bufs=14) as e_pool,
            tc.tile_pool(name="attn", bufs=1) as attn_pool,
            tc.tile_pool(name="norm", bufs=1) as norm_pool,
            tc.tile_pool(name="ysb", bufs=4) as y_pool,
            tc.tile_pool(name="dram", bufs=1, space="DRAM") as dram_pool,
            tc.tile_pool(name="big_ps", bufs=2, space="PSUM") as big_ps,
            tc.tile_pool(name="st_ps", bufs=2, space="PSUM") as st_ps,
        ):
            # ---- inputs: x/pe first (they gate everything), spread queues ----
            x_t, pe_t = [], []
            for ct in range(CT):
                xt = xin_pool.tile([128, N], BF16, tag=f"x{ct}")
                pt = consts.tile([128, N], BF16, tag=f"pe{ct}")
                eng = nc.sync if ct % 2 == 0 else nc.scalar
                eng2 = nc.scalar if ct % 2 == 0 else nc.sync
                eng.dma_start(xt[:], x_ext[128 * ct : 128 * (ct + 1), :])
                eng2.dma_start(pt[:], pe_ext[128 * ct : 128 * (ct + 1), :])
                x_t.append(xt)
                pe_t.append(pt)

            xpe_bf = []
            for ct in range(CT):
                xb = xpe_pool.tile([128, N], BF16, tag=f"xpe{ct}")
                nc.vector.tensor_tensor(
                    out=xb[:], in0=x_t[ct][:], in1=pe_t[ct][:], op=mybir.AluOpType.add
                )
                xpe_bf.append(xb)

            wq_t, wk_t, wv_t, wo_t = [], [], [], []
            for lst, ext, nm in (
                (wq_t, wqT_ext, "wq"),
                (wk_t, wkT_ext, "wk"),
                (wv_t, wvT_ext, "wv"),
                (wo_t, woT_ext, "wo"),
            ):
                for ct in range(CT):
                    t = consts.tile([128, C], BF16, tag=f"{nm}{ct}")
                    nc.scalar.dma_start(t[:], ext[128 * ct : 128 * (ct + 1), :])
                    lst.append(t)
            bo_t = []
            for ct in range(CT):
                t = consts.tile([128, 1], F32, tag=f"bo{ct}")
                nc.scalar.dma_start(t[:], bo_ext[128 * ct : 128 * (ct + 1), :])
                bo_t.append(t)
            em_sb = consts.tile([2, 128], F32, tag="em")
            nc.scalar.dma_start(em_sb[:], em_ext[:])

            def proj_into(w_t, dst, rhs_of):
                """dst[ct][:, :] = (w.T @ rhs) for each 128-row output tile.

                Weight slices stay stationary for both 512-wide chunks.
                """
                for ct in range(CT):
                    ps = [big_ps.tile([128, NC_], F32, tag="big", name=f"projps{i}", padded_shape=[128, 2 * NC_]) for i in range(2)]
                    for kc in range(CT):
                        for nch in range(2):
                            nc.tensor.matmul(
                                ps[nch][:],
                                w_t[kc][:, 128 * ct : 128 * (ct + 1)],
                                rhs_of(kc, nch),
                                start=(kc == 0),
                                stop=(kc == CT - 1),
                            )
                    for nch in range(2):
                        nc.vector.tensor_copy(
                            dst[ct][:, NC_ * nch : NC_ * (nch + 1)], ps[nch][:]
                        )

            # ---- projections: q, k  [512, 1024] bf16, channel-on-partition ----
            q_bf = [qk_pool.tile([128, N], BF16, tag=f"q{c}", name=f"q{c}") for c in range(CT)]
            k_bf = [qk_pool.tile([128, N], BF16, tag=f"k{c}", name=f"k{c}") for c in range(CT)]
            proj_into(
                wq_t, q_bf, lambda kc, nch: xpe_bf[kc][:, NC_ * nch : NC_ * (nch + 1)]
            )
            proj_into(
                wk_t, k_bf, lambda kc, nch: xpe_bf[kc][:, NC_ * nch : NC_ * (nch + 1)]
            )

            # ---- v^T with ones columns: [1024, 8*65] bf16, token-on-partition ----
            vt_bf = []
            for mt in range(MT):
                vt = vt_pool.tile([128, NH * (D + 1)], BF16, tag=f"vt{mt}")
                nc.gpsimd.memset(vt[:], 1.0)
                ps = big_ps.tile([128, NC_], F32, tag="big", padded_shape=[128, 2 * NC_])
                for kc in range(CT):
                    nc.tensor.matmul(
                        ps[:],
                        xpe_bf[kc][:, 128 * mt : 128 * (mt + 1)],
                        wv_t[kc][:],
                        start=(kc == 0),
                        stop=(kc == CT - 1),
                    )
                hv = vt[:].rearrange("p (h e) -> p h e", e=D + 1)
                nc.vector.tensor_copy(
                    hv[:, :, 0:D],
                    ps[:].rearrange("p (h e) -> p h e", e=D),
                )
                vt_bf.append(vt)

            # ---- attention (PV of the previous head fills PE gaps while the
            # current head's scores wait on exp) ----
            attn_f32 = [
                attn_pool.tile([128, N], F32, tag=f"attn{ct}", name=f"attn{ct}")
                for ct in range(CT)
            ]
            attn_bf = [
                attn_pool.tile([128, N], BF16, tag=f"attnbf{ct}", name=f"attnbf{ct}")
                for ct in range(CT)
            ]
            recip_dram = dram_pool.tile([NH, N], F32, tag="recipd")
            units = [(hp, half) for hp in range(NH // 2) for half in range(2)]
            e_of = {}
            pv_of = {}

            def issue_pv_mt(u, mt):
                hp, half = u
                h = 2 * hp + half
                for nch in range(2):
                    nc.tensor.matmul(
                        pv_of[u][:, NC_ * nch : NC_ * (nch + 1)],
                        vt_bf[mt][:, (D + 1) * h : (D + 1) * (h + 1)],
                        e_of[u][mt][:, NC_ * nch : NC_ * (nch + 1)],
                        start=(mt == 0),
                        stop=(mt == MT - 1),
                    )

            def drain_pv(u, denom_pairs):
                hp, half = u
                ct = hp
                lo = D * half
                # PSUM isn't DMA-readable: copy to SBUF, then DMA into the
                # stacked layout (partition shift) + denominator row
                tmp = y_pool.tile([D + 1, 2 * NC_], F32, tag="pvtmp", bufs=3)
                nc.vector.tensor_copy(tmp[:], pv_of[u][:])
                nc.sync.dma_start(attn_f32[ct][lo : lo + D, :], tmp[0:D, :])
                nc.sync.dma_start(
                    denom_pairs[hp][half : half + 1, :], tmp[D : D + 1, :]
                )
                if half == 1:
                    rp = norm_pool.tile([2, N], F32, tag=f"rec{hp}", name=f"rec{hp}")
                    nc.vector.reciprocal_approx_fast(rp[:], denom_pairs[hp][:])
                    if hp < NH // 2 - 1:
                        nc.sync.dma_start(recip_dram[2 * hp : 2 * hp + 2, :], rp[:])
                        bc = attn_pool.tile(
                            [128, N], F32, tag=f"bcast{hp}", name=f"bcast{hp}"
                        )
                        for bh in range(2):
                            nc.sync.dma_start(
                                bc[D * bh : D * (bh + 1), :],
                                recip_dram[
                                    2 * hp + bh : 2 * hp + bh + 1, :
                                ].to_broadcast((D, N)),
                            )
                        nc.vector.tensor_tensor(
                            out=attn_bf[ct][:], in0=attn_f32[ct][:], in1=bc[:],
                            op=mybir.AluOpType.mult,
                        )
                    else:
                        # tail: broadcast on the PE (fp32 E-matrix matmul) so
                        # the array stays busy through the normalize chain
                        for nch in range(2):
                            bc_ps = st_ps.tile(
                                [128, NC_], F32, tag="st", name=f"bcps{nch}"
                            )
                            nc.tensor.matmul(
                                bc_ps[:],
                                em_sb[:],
                                rp[:, NC_ * nch : NC_ * (nch + 1)],
                                start=True,
                                stop=True,
                            )
                            nc.vector.tensor_tensor(
                                out=attn_bf[ct][:, NC_ * nch : NC_ * (nch + 1)],
                                in0=attn_f32[ct][:, NC_ * nch : NC_ * (nch + 1)],
                                in1=bc_ps[:],
                                op=mybir.AluOpType.mult,
                            )

            denom_pairs = [
                norm_pool.tile([2, N], F32, tag=f"den{hp}", name=f"den{hp}")
                for hp in range(NH // 2)
            ]
            prev = None
            for u in units:
                hp, half = u
                ct = hp
                lo = D * half
                e_of[u] = []
                pv_of[u] = big_ps.tile(
                    [D + 1, 2 * NC_], F32, tag="big", name=f"pv{hp}_{half}"
                )
                for mt in range(MT):
                    st = st_ps.tile([128, 2 * NC_], F32, tag="st")
                    for nch in range(2):
                        nc.tensor.matmul(
                            st[:, NC_ * nch : NC_ * (nch + 1)],
                            k_bf[ct][lo : lo + D, 128 * mt : 128 * (mt + 1)],
                            q_bf[ct][lo : lo + D, NC_ * nch : NC_ * (nch + 1)],
                            start=True,
                            stop=True,
                        )
                    e_t = e_pool.tile([128, 2 * NC_], BF16, tag="e")
                    nc.scalar.activation(
                        e_t[:], st[:], mybir.ActivationFunctionType.Exp,
                        scale=SCALE,
                    )
                    e_of[u].append(e_t)
                    if prev is not None:
                        issue_pv_mt(prev, mt)
                if prev is not None:
                    drain_pv(prev, denom_pairs)
                    del e_of[prev]
                prev = u
            for mt in range(MT):
                issue_pv_mt(prev, mt)
            drain_pv(prev, denom_pairs)

            # ---- output projection + bias ----
            for ct in range(CT):
                ps = [big_ps.tile([128, NC_], F32, tag="big", name=f"yps{i}", padded_shape=[128, 2 * NC_]) for i in range(2)]
                for kc in range(CT):
                    for nch in range(2):
                        nc.tensor.matmul(
                            ps[nch][:],
                            wo_t[kc][:, 128 * ct : 128 * (ct + 1)],
                            attn_bf[kc][:, NC_ * nch : NC_ * (nch + 1)],
                            start=(kc == 0),
                            stop=(kc == CT - 1),
                        )
                for nch in range(2):
                    yt = y_pool.tile([128, NC_], F32, tag="y")
                    nc.vector.tensor_scalar_add(yt[:], ps[nch][:], bo_t[ct][:])
                    nc.sync.dma_start(
                        y_ext[128 * ct : 128 * (ct + 1), NC_ * nch : NC_ * (nch + 1)],
                        yt[:],
                    )

    nc.compile()
    return nc


_PROGRAM = None


def make_in_maps(x, wq, wk, wv, wo, bo):
    bf = ml_dtypes.bfloat16
    pe = _pe_table().astype(bf)
    wqT = np.ascontiguousarray(np.asarray(wq).T).astype(bf)
    wkT = np.ascontiguousarray(np.asarray(wk).T).astype(bf)
    wvT = np.ascontiguousarray(np.asarray(wv).T).astype(bf)
    woT = np.ascontiguousarray(np.asarray(wo).T).astype(bf)
    bo2 = np.ascontiguousarray(np.asarray(bo, dtype=np.float32).reshape(C, 1))
    x = np.asarray(x, dtype=np.float32)
    em = np.zeros((2, 128), dtype=np.float32)
    em[0, 0:D] = 1.0
    em[1, D : 2 * D] = 1.0

    return [
        {
            "x": np.ascontiguousarray(x[b].reshape(C, N)).astype(bf),
            "pe": pe,
            "wqT": wqT,
            "wkT": wkT,
            "wvT": wvT,
            "woT": woT,
            "bo": bo2,
            "ematrix": em,
        }
        for b in range(B)
    ]


def kernel(x, wq, wk, wv, wo, bo):
    global _PROGRAM
    if _PROGRAM is None:
        _PROGRAM = _build_program()
    nc = _PROGRAM

    in_maps = make_in_maps(x, wq, wk, wv, wo, bo)
    res = run_bass_kernel_spmd(nc, in_maps, list(range(B)))
    out = np.stack([res.results[b]["y"].reshape(C, H, W) for b in range(B)])
    return out.astype(np.float32)

